# revision 7
# baseline (speedup 1.0000x reference)
"""AggrGATConv Trainium2 kernel v4: int16 message stream, globally-chunked
weight chain, engine-split identity aggregation.

Per-core (dst-sharded identity layout):
  inv-1: h = feat @ W (fp32 2-pass exact); q = round(h * 32767/max|h|) int16;
    sg = max|h|/(32767*H) (1/H head-mean folded in); el/er tables.
  host (index/data movement only): deg-sorted octet slotting; gathers
    q16[src] into per-window blobs, el[src]/sg[src] streams, er replicated
    per slot (erx).  All weight streams (t,h)-major per window.
  inv-2:
    global chain (chunked contiguous ops): lg = el + erx; e1 = exp(lg);
      e2 = exp(.2 lg); ee = max(e1,e2); esg = ee * sg
    per T-group: s4 = sum_t ee; r4 = 1/s4; es = esg * r4_bc
    per subgroup (<=4 same-T windows): one q DMA; PE-red subgroups get one
      merged mult (DVE or GpSimd); DVE-red windows get per-window mults.
    reduce: PE 1-tile identity matmuls + [P,D,H] X-reduce,  or fused
      XY-reduce on DVE.  Outputs accumulate in outbuf; one +bias; one DMA.
"""
import sys
import types
import contextlib
import ctypes
import os

import numpy as np

import concourse.bacc as bacc
import concourse.tile as tile
import concourse.mybir as mybir
from concourse.bass_utils import run_bass_kernel_spmd

# ---------------- constants (hardcoded per problem spec) ----------------
N = 100000
E = 1600000
IN = 128
H, D = 4, 32
HD = H * D  # 128
NEG = 0.2
NCORES = 8
P = 128
K_WIN = 98
N_PAD = NCORES * K_WIN * P       # 100352
NODES_PER_CORE = K_WIN * P       # 12544
PAD_LOGIT = -100.0
FLOOR_LOGIT = -69.07755  # 5*ln(1e-6): keeps s4 >= 1e-6 on padded rows
QMAX = 32767.0

SUBG_TILES = 20     # max total tiles per subgroup
PE_RED_FRAC = 0.68  # fraction of windows with PE reduce
GPS_MULT_FRAC = 0.60  # fraction of windows with GpSimd mult
CHAIN_CHUNK = 700   # target elems/partition per global-chain chunk

f32 = mybir.dt.float32
i16 = mybir.dt.int16

Exp = mybir.ActivationFunctionType.Exp
Copy = mybir.ActivationFunctionType.Copy
Add = mybir.AluOpType.add
Mult = mybir.AluOpType.mult
Max = mybir.AluOpType.max


def _install_ntff_shim():
    if "antenv.axon_hooks" in sys.modules:
        return
    try:
        lib = ctypes.CDLL("/opt/axon/libaxon_pjrt.so")
        if not hasattr(lib, "axon_start_nrt_profile"):
            raise OSError("no symbol")
        lib.axon_start_nrt_profile.argtypes = [
            ctypes.POINTER(ctypes.c_int64), ctypes.c_size_t]
        lib.axon_start_nrt_profile.restype = ctypes.c_int64
        lib.axon_stop_nrt_profile.argtypes = [ctypes.c_char_p]
        lib.axon_stop_nrt_profile.restype = ctypes.c_int64

        @contextlib.contextmanager
        def _hook(output_dir, device_ids):
            import jax
            jax.devices()
            if device_ids:
                ids = (ctypes.c_int64 * len(device_ids))(*device_ids)
                rc = lib.axon_start_nrt_profile(ids, len(device_ids))
            else:
                rc = lib.axon_start_nrt_profile(None, 0)
            if rc != 0:
                raise RuntimeError(f"axon_start_nrt_profile rc={rc}")
            try:
                yield
            finally:
                n = lib.axon_stop_nrt_profile(str(output_dir).encode())
                print(f"profile: {n} file(s) -> {output_dir}", file=sys.stderr)

        hook = _hook
    except OSError:
        hook = None
    mod = types.ModuleType("antenv.axon_hooks")
    mod.get_axon_ntff_profile_hook = lambda: hook
    mod.set_axon_ntff_profile_hook = lambda h: None
    sys.modules["antenv.axon_hooks"] = mod


_install_ntff_shim()


# ---------------- invocation 1: node tables + int16 quantization ---------
def _build_inv1():
    nc = bacc.Bacc("TRN2", target_bir_lowering=False, debug=False,
                   num_devices=NCORES)
    featT = nc.declare_dram_parameter("featT", [P, NODES_PER_CORE], f32,
                                      isOutput=False)
    W_in = nc.declare_dram_parameter("W", [IN, HD], f32, isOutput=False)
    WT_in = nc.declare_dram_parameter("WT", [HD, IN], f32, isOutput=False)
    Al_in = nc.declare_dram_parameter("Al", [HD, 4], f32, isOutput=False)
    Ar_in = nc.declare_dram_parameter("Ar", [HD, 4], f32, isOutput=False)
    q_out = nc.declare_dram_parameter("q_out", [P, K_WIN * HD], i16,
                                      isOutput=True)
    elr_out = nc.declare_dram_parameter("elr_out", [P, K_WIN * 8], f32,
                                        isOutput=True)
    sg_out = nc.declare_dram_parameter("sg_out", [P, K_WIN * 4], f32,
                                       isOutput=True)

    with tile.TileContext(nc) as tc:
        with tc.tile_pool(name="cst", bufs=1) as cst, \
             tc.tile_pool(name="sb", bufs=3) as sb, \
             tc.tile_pool(name="ps", bufs=3, space="PSUM") as ps, \
             tc.tile_pool(name="psw", bufs=1, space="PSUM") as psw:

            wt_sb = cst.tile([HD, IN], f32, tag="wt")
            nc.sync.dma_start(out=wt_sb[:], in_=WT_in[:])
            al_sb = cst.tile([HD, 4], f32, tag="al")
            nc.sync.dma_start(out=al_sb[:], in_=Al_in[:])
            ar_sb = cst.tile([HD, 4], f32, tag="ar")
            nc.sync.dma_start(out=ar_sb[:], in_=Ar_in[:])

            wlr = cst.tile([IN, 136], f32, tag="wlr")
            nc.sync.dma_start(out=wlr[:, 0:HD], in_=W_in[:])
            wl_ps = psw.tile([IN, 8], f32, tag="wlp")
            nc.tensor.matmul(out=wl_ps[:, 0:4], lhsT=wt_sb[:], rhs=al_sb[:],
                             start=True, stop=True)
            nc.tensor.matmul(out=wl_ps[:, 4:8], lhsT=wt_sb[:], rhs=ar_sb[:],
                             start=True, stop=True)
            nc.scalar.activation(out=wlr[:, 128:136], in_=wl_ps[:],
                                 func=Copy)

            CH = 14
            n_chunks = NODES_PER_CORE // (P * CH)
            for c in range(n_chunks):
                ft = sb.tile([P, CH * P], f32, tag="ft")
                nc.sync.dma_start(
                    out=ft[:], in_=featT[:, c * CH * P:(c + 1) * CH * P])
                hsb = sb.tile([P, CH * 136], f32, tag="hsb")
                for t in range(CH):
                    hp = ps.tile([P, 136], f32, tag="hp")
                    nc.tensor.matmul(out=hp[:],
                                     lhsT=ft[:, t * P:(t + 1) * P],
                                     rhs=wlr[:], start=True, stop=True)
                    if t % 2 == 0:
                        nc.scalar.activation(
                            out=hsb[:, t * 136:(t + 1) * 136], in_=hp[:],
                            func=Copy)
                    else:
                        nc.vector.tensor_copy(hsb[:, t * 136:(t + 1) * 136],
                                              hp[:])
                ga = hsb[:].rearrange("p (c f) -> p c f", c=CH)
                hview = ga[:, :, 0:128].rearrange(
                    "p c (hh d) -> p c hh d", hh=H)
                m4 = sb.tile([P, CH * 4], f32, tag="m4")
                nc.vector.tensor_reduce(
                    out=m4[:].rearrange("p (c h) -> p c h", c=CH),
                    in_=hview, axis=mybir.AxisListType.X, op=Max,
                    apply_absolute_value=True)
                # sg table carries the 1/H fold for the head-mean
                sgc = sb.tile([P, CH * 4], f32, tag="sgc")
                nc.vector.tensor_scalar_mul(sgc[:], m4[:], 1.0 / (QMAX * H))
                minv = sb.tile([P, CH * 4], f32, tag="minv")
                nc.vector.reciprocal(minv[:], m4[:])
                sinv = sb.tile([P, CH * 4], f32, tag="sinv")
                nc.vector.tensor_scalar_mul(sinv[:], minv[:], QMAX)
                q16 = sb.tile([P, CH * HD], i16, tag="q16")
                nc.vector.tensor_tensor(
                    out=q16[:].rearrange("p (c hh d) -> p c hh d",
                                         c=CH, hh=H),
                    in0=hview,
                    in1=sinv[:].rearrange("p (c h) -> p c h", c=CH)
                        .unsqueeze(3).to_broadcast([P, CH, H, D]),
                    op=Mult)
                elrs = sb.tile([P, CH * 8], f32, tag="elrs")
                nc.gpsimd.tensor_copy(
                    elrs[:].rearrange("p (c e) -> p c e", c=CH),
                    ga[:, :, 128:136])
                nc.gpsimd.dma_start(
                    out=q_out[:, c * CH * HD:(c + 1) * CH * HD], in_=q16[:])
                nc.gpsimd.dma_start(
                    out=elr_out[:, c * CH * 8:(c + 1) * CH * 8], in_=elrs[:])
                nc.gpsimd.dma_start(
                    out=sg_out[:, c * CH * 4:(c + 1) * CH * 4], in_=sgc[:])
    nc.compile()
    return nc


def _schedule(Ts):
    """ASC-T schedule with equal-T groups, subgroups, engine classes and
    stream offsets.  All layouts (t,h)-major per window."""
    Ts = list(Ts)
    order = [int(x) for x in np.argsort(np.asarray(Ts), kind="stable")]
    groups = []
    i = 0
    while i < len(order):
        j = i
        tval = Ts[order[i]]
        while j < len(order) and Ts[order[j]] == tval:
            j += 1
        groups.append((int(tval), order[i:j]))
        i = j
    # stream offsets in scheduled order
    q_off, a_off = {}, {}
    qo = ao = 0
    for w in order:
        q_off[w] = qo
        qo += Ts[w] * HD
        a_off[w] = ao
        ao += 4 * Ts[w]
    A4 = ao
    # subgroups + engine classes (window-count-weighted ratio targets)
    sub_of = []           # list of (T, [windows], use_pe, use_gps)
    npe = ngps = ntot = 0
    for (tval, ws) in groups:
        k = 0
        nwin = max(1, SUBG_TILES // tval)
        while k < len(ws):
            sub = ws[k:k + nwin]
            k += nwin
            use_pe = (npe <= PE_RED_FRAC * ntot)
            use_gps = (ngps <= GPS_MULT_FRAC * ntot)
            ntot += len(sub)
            if use_pe:
                npe += len(sub)
            if use_gps:
                ngps += len(sub)
            sub_of.append((tval, sub, use_pe, use_gps))
    use_pe_w = {}
    for (tval, sub, upe, ugps) in sub_of:
        for w in sub:
            use_pe_w[w] = upe
    # chain chunks: consecutive groups totalling <= CHAIN_CHUNK elems/part
    chunks = []
    cur = []
    cur_elems = 0
    for (tval, ws) in groups:
        gsz = 4 * tval * len(ws)
        if cur and cur_elems + gsz > CHAIN_CHUNK:
            chunks.append(cur)
            cur = []
            cur_elems = 0
        cur.append((tval, ws))
        cur_elems += gsz
    if cur:
        chunks.append(cur)
    return dict(order=order, groups=groups, q_off=q_off, a_off=a_off,
                A4=A4, CAPQ=qo, subs=sub_of, use_pe=use_pe_w,
                chunks=chunks)


# ---------------- invocation 2: edge aggregation ----------------
def _build_inv2(Ts):
    meta = _schedule(Ts)
    order = meta["order"]
    q_off, a_off, A4, CAPQ = (meta["q_off"], meta["a_off"], meta["A4"],
                              meta["CAPQ"])
    nsched = len(order)
    sched_pos = {w: i for i, w in enumerate(order)}

    nc = bacc.Bacc("TRN2", target_bir_lowering=False, debug=False,
                   num_devices=NCORES)
    q_d = nc.declare_dram_parameter("q", [P, CAPQ], i16, isOutput=False)
    el_d = nc.declare_dram_parameter("el", [P, A4], f32, isOutput=False)
    er_d = nc.declare_dram_parameter("er", [P, A4], f32, isOutput=False)
    sg_d = nc.declare_dram_parameter("sg", [P, A4], f32, isOutput=False)
    ident_d = nc.declare_dram_parameter("ident", [P, P], f32, isOutput=False)
    bias_in = nc.declare_dram_parameter("bias", [1, HD], f32, isOutput=False)
    out_d = nc.declare_dram_parameter("out", [P, K_WIN * D], f32,
                                      isOutput=True)

    with tile.TileContext(nc) as tc:
        with tc.tile_pool(name="cst", bufs=1) as cst, \
             tc.tile_pool(name="ax", bufs=2) as ax, \
             tc.tile_pool(name="ch", bufs=2) as chp, \
             tc.tile_pool(name="ld", bufs=3) as ld, \
             tc.tile_pool(name="wk", bufs=2) as wk, \
             tc.tile_pool(name="fl", bufs=2) as fl, \
             tc.tile_pool(name="ps", bufs=4, space="PSUM") as ps, \
             tc.tile_pool(name="psb", bufs=1, space="PSUM") as psb:

            ident = cst.tile([P, P], f32, tag="ident")
            nc.sync.dma_start(out=ident[:], in_=ident_d[:])

            bias_sb = cst.tile([1, HD], f32, tag="brow")
            nc.sync.dma_start(out=bias_sb[:], in_=bias_in[:])
            bias_m = cst.tile([1, D], f32, tag="bm")
            nc.vector.tensor_reduce(
                out=bias_m[:],
                in_=bias_sb[0:1, :].rearrange("p (h d) -> p d h", h=H),
                axis=mybir.AxisListType.X, op=Add)
            nc.vector.tensor_scalar_mul(bias_m[:], bias_m[:], 1.0 / H)
            ones1 = cst.tile([1, P], f32, tag="ones")
            nc.vector.memset(ones1[:], 1.0)
            bias_ps = psb.tile([P, D], f32, tag="bps")
            nc.tensor.matmul(out=bias_ps[:], lhsT=ones1[:], rhs=bias_m[:],
                             start=True, stop=True)
            bias_bc = cst.tile([P, D], f32, tag="bbc")
            nc.vector.tensor_copy(bias_bc[:], bias_ps[:])

            # resident final weights + outputs
            es_all = cst.tile([P, A4], f32, tag="es_all")
            outbuf = cst.tile([P, nsched * D], f32, tag="outbuf")

            # ---- stage 1: global weight chain, chunked ----
            for chunk in meta["chunks"]:
                c_lo = a_off[chunk[0][1][0]]
                c_sz = sum(4 * t * len(ws) for (t, ws) in chunk)
                elt = ax.tile([P, c_sz], f32, tag="elt")
                nc.sync.dma_start(out=elt[:], in_=el_d[:, c_lo:c_lo + c_sz])
                ert = ax.tile([P, c_sz], f32, tag="ert")
                nc.sync.dma_start(out=ert[:], in_=er_d[:, c_lo:c_lo + c_sz])
                sgt = ax.tile([P, c_sz], f32, tag="sgt")
                nc.sync.dma_start(out=sgt[:], in_=sg_d[:, c_lo:c_lo + c_sz])

                lg = chp.tile([P, c_sz], f32, tag="lg")
                nc.vector.tensor_tensor(out=lg[:], in0=elt[:], in1=ert[:],
                                        op=Add)
                e1 = chp.tile([P, c_sz], f32, tag="e1")
                nc.scalar.activation(out=e1[:], in_=lg[:], func=Exp)
                e2 = chp.tile([P, c_sz], f32, tag="e2")
                nc.scalar.activation(out=e2[:], in_=lg[:], scale=NEG,
                                     func=Exp)
                ee = chp.tile([P, c_sz], f32, tag="ee")
                nc.vector.tensor_tensor(out=ee[:], in0=e1[:], in1=e2[:],
                                        op=Max)
                esg = chp.tile([P, c_sz], f32, tag="esg")
                nc.vector.tensor_tensor(out=esg[:], in0=ee[:], in1=sgt[:],
                                        op=Mult)

                # ---- stage 2 per T-group in chunk: s4, r4, es ----
                for (T, ws) in chunk:
                    nk = len(ws)
                    g0 = a_off[ws[0]] - c_lo
                    gsz = 4 * T * nk
                    s4 = fl.tile([P, nk * 4], f32, tag="s4")
                    nc.vector.tensor_reduce(
                        out=s4[:].rearrange("p (k h) -> p k h", k=nk),
                        in_=ee[:, g0:g0 + gsz].rearrange(
                            "p (k t h) -> p k h t", k=nk, h=H),
                        axis=mybir.AxisListType.X, op=Add)
                    r4 = fl.tile([P, nk * 4], f32, tag="r4")
                    nc.vector.reciprocal(r4[:], s4[:])
                    nc.vector.tensor_tensor(
                        out=es_all[:, a_off[ws[0]]:a_off[ws[0]] + gsz]
                            .rearrange("p (k t h) -> p k t h", k=nk, h=H),
                        in0=esg[:, g0:g0 + gsz].rearrange(
                            "p (k t h) -> p k t h", k=nk, h=H),
                        in1=r4[:].rearrange("p (k h) -> p k h", k=nk)
                            .unsqueeze(2).to_broadcast([P, nk, T, H]),
                        op=Mult)

            # ---- stage 3: per subgroup heavy ops ----
            for (T, sub, use_pe, use_gps) in meta["subs"]:
                ns = len(sub)
                KW = T * HD
                qt = ld.tile([P, ns * KW], i16, tag="qt")
                nc.sync.dma_start(
                    out=qt[:],
                    in_=q_d[:, q_off[sub[0]]:q_off[sub[0]] + ns * KW])
                meng = nc.gpsimd if use_gps else nc.vector
                e0 = a_off[sub[0]]
                if use_pe:
                    # merged mult across the subgroup; layout (t,h,d)
                    wmsg = wk.tile([P, ns * KW], f32, tag="wmsg")
                    meng.tensor_tensor(
                        out=wmsg[:].rearrange("p (kt hh d) -> p kt hh d",
                                              hh=H, d=D),
                        in0=qt[:].rearrange("p (kt hh d) -> p kt hh d",
                                            hh=H, d=D),
                        in1=es_all[:, e0:e0 + ns * 4 * T]
                            .rearrange("p (kt h) -> p kt h", h=H)
                            .unsqueeze(3).to_broadcast([P, ns * T, H, D]),
                        op=Mult)
                    for si, w in enumerate(sub):
                        u = ps.tile([P, HD], f32, tag="u")
                        for t in range(T):
                            nc.tensor.matmul(
                                out=u[:], lhsT=ident[:],
                                rhs=wmsg[:, (si * T + t) * HD:
                                         (si * T + t + 1) * HD],
                                start=(t == 0), stop=(t == T - 1))
                        sp = sched_pos[w]
                        nc.vector.tensor_reduce(
                            out=outbuf[:, sp * D:(sp + 1) * D],
                            in_=u[:].rearrange("p (hh d) -> p d hh", hh=H),
                            axis=mybir.AxisListType.X, op=Add)
                else:
                    # per-window mult (h,d,t) + fused XY reduce on DVE
                    for si, w in enumerate(sub):
                        wmsg = wk.tile([P, KW], f32, tag="wmsg")
                        meng.tensor_tensor(
                            out=wmsg[:].rearrange("p (hh d t) -> p hh d t",
                                                  hh=H, d=D),
                            in0=qt[:, si * KW:(si + 1) * KW]
                                .rearrange("p (hh d t) -> p hh d t",
                                           hh=H, d=D),
                            in1=es_all[:, e0 + si * 4 * T:
                                       e0 + (si + 1) * 4 * T]
                                .rearrange("p (t h) -> p h t", h=H)
                                .unsqueeze(2).to_broadcast([P, H, D, T]),
                            op=Mult)
                        sp = sched_pos[w]
                        nc.vector.tensor_reduce(
                            out=outbuf[:, sp * D:(sp + 1) * D],
                            in_=wmsg[:].rearrange("p (hh d t) -> p d hh t",
                                                  hh=H, d=D),
                            axis=mybir.AxisListType.XY, op=Add)

            # ---- finalize ----
            nc.vector.tensor_tensor(
                out=outbuf[:].rearrange("p (k d) -> p k d", k=nsched),
                in0=outbuf[:].rearrange("p (k d) -> p k d", k=nsched),
                in1=bias_bc[:].unsqueeze(1).to_broadcast([P, nsched, D]),
                op=Add)
            nc.gpsimd.dma_start(out=out_d[:, 0:nsched * D], in_=outbuf[:])
    nc.compile()
    return nc, meta


_INV1 = None
_INV2 = {}
LAST_EXEC_NS = None
LAST_EXEC_NS1 = None
LAST_EXEC_NS2 = None
_TRACE = bool(os.environ.get("GAT_TRACE"))


def kernel(feat, W, attn_l, attn_r, bias, src, dst):
    global _INV1, LAST_EXEC_NS, LAST_EXEC_NS1, LAST_EXEC_NS2
    feat = np.asarray(feat, dtype=np.float32)
    W = np.asarray(W, dtype=np.float32)
    attn_l = np.asarray(attn_l, dtype=np.float32)
    attn_r = np.asarray(attn_r, dtype=np.float32)
    bias = np.asarray(bias, dtype=np.float32)
    src = np.asarray(src, dtype=np.int32)
    dst = np.asarray(dst, dtype=np.int32)

    featT = np.zeros((IN, N_PAD), dtype=np.float32)
    featT[:, :N] = np.ascontiguousarray(feat.T)
    WT = np.ascontiguousarray(W.T)
    Al = np.zeros((HD, H), dtype=np.float32)
    Ar = np.zeros((HD, H), dtype=np.float32)
    for h in range(H):
        Al[h * D:(h + 1) * D, h] = attn_l[h]
        Ar[h * D:(h + 1) * D, h] = attn_r[h]

    # ---------------- inv-1 ----------------
    if _INV1 is None:
        _INV1 = _build_inv1()
    in1 = []
    for c in range(NCORES):
        sl = slice(c * NODES_PER_CORE, (c + 1) * NODES_PER_CORE)
        in1.append({"featT": np.ascontiguousarray(featT[:, sl]),
                    "W": W, "WT": WT, "Al": Al, "Ar": Ar})
    res1 = run_bass_kernel_spmd(_INV1, in1, core_ids=list(range(NCORES)),
                                trace=_TRACE)
    LAST_EXEC_NS1 = res1.exec_time_ns
    q_full = np.concatenate(
        [r["q_out"].reshape(P, K_WIN, HD).transpose(1, 0, 2)
         .reshape(NODES_PER_CORE, HD) for r in res1.results], axis=0)
    elr_full = np.concatenate(
        [r["elr_out"].reshape(P, K_WIN, 8).transpose(1, 0, 2)
         .reshape(NODES_PER_CORE, 8) for r in res1.results], axis=0)
    sg_full = np.concatenate(
        [r["sg_out"].reshape(P, K_WIN, 4).transpose(1, 0, 2)
         .reshape(NODES_PER_CORE, 4) for r in res1.results], axis=0)

    # ---------------- host: identity-layout slotting ----------------
    deg = np.bincount(dst, minlength=N_PAD).astype(np.int64)
    order_n = np.argsort(-deg, kind="stable")
    rank = np.empty(N_PAD, dtype=np.int64)
    rank[order_n] = np.arange(N_PAD)
    k_of = rank >> 10
    within = rank & 1023
    c_of = within >> 7
    c_of = np.where(k_of & 1 == 1, NCORES - 1 - c_of, c_of)  # snake
    p_of = within & 127

    Ts = deg[order_n[::1024]]
    Ts = np.maximum(Ts, 1)
    key = tuple(int(t) for t in Ts)
    if key not in _INV2:
        _INV2[key] = _build_inv2(key)
    nc2, meta = _INV2[key]

    Ts_np = np.asarray(key, dtype=np.int64)
    nsched = len(meta["order"])
    sched_pos = np.empty(K_WIN, dtype=np.int64)
    sched_pos[np.asarray(meta["order"])] = np.arange(nsched)
    q_off = np.zeros(K_WIN, dtype=np.int64)
    a_off = np.zeros(K_WIN, dtype=np.int64)
    use_pe_w = np.zeros(K_WIN, dtype=bool)
    for w in range(K_WIN):
        q_off[w] = meta["q_off"][w]
        a_off[w] = meta["a_off"][w]
        use_pe_w[w] = meta["use_pe"][w]
    CAPQ, A4 = meta["CAPQ"], meta["A4"]

    # per-edge slots
    perm = np.argsort(dst, kind="stable")
    dstp = dst[perm]
    srcp = src[perm]
    estart = np.zeros(N_PAD + 1, dtype=np.int64)
    np.cumsum(np.bincount(dstp, minlength=N_PAD), out=estart[1:])
    te = np.arange(E, dtype=np.int64) - estart[dstp]
    ce = c_of[dstp]
    pe_row = p_of[dstp]
    we = k_of[dstp]
    Te = Ts_np[we]

    # ---- q stream ----
    q_lay = np.zeros((NCORES, P, CAPQ), dtype=np.int16)
    qflat = q_lay.reshape(-1)
    rowbase = (ce * P + pe_row) * CAPQ
    hdidx = np.arange(HD, dtype=np.int64)
    is_pe_e = use_pe_w[we]
    idx_pe = np.nonzero(is_pe_e)[0]
    idx_dv = np.nonzero(~is_pe_e)[0]
    # PE windows: (t, h, d) -> cols q_off + t*128 + hd
    cols = (rowbase[idx_pe] + q_off[we[idx_pe]]
            + te[idx_pe] * HD)[:, None] + hdidx[None, :]
    qflat[cols] = q_full[srcp[idx_pe]]
    del cols
    # DVE windows: (h, d, t) -> cols q_off + hd*T + t
    cols = (rowbase[idx_dv] + q_off[we[idx_dv]] + te[idx_dv])[:, None] \
        + hdidx[None, :] * Te[idx_dv][:, None]
    qflat[cols] = q_full[srcp[idx_dv]]
    del cols

    # ---- weight streams (t,h)-major ----
    el_lay = np.full((NCORES, P, A4), PAD_LOGIT, dtype=np.float32)
    er_lay = np.zeros((NCORES, P, A4), dtype=np.float32)
    sg_lay = np.zeros((NCORES, P, A4), dtype=np.float32)
    hidx = np.arange(H, dtype=np.int64)
    rb_a = (ce * P + pe_row) * A4
    ecols = (rb_a + a_off[we] + te * 4)[:, None] + hidx[None, :]
    el_lay.reshape(-1)[ecols] = elr_full[srcp][:, 0:4]
    sg_lay.reshape(-1)[ecols] = sg_full[srcp]
    del ecols
    # er replicated across all slots of each row (including pads: harmless)
    nodes = np.arange(N_PAD)
    for w in range(K_WIN):
        rows = nodes[k_of == w]
        Tw = int(Ts_np[w])
        erv = elr_full[rows][:, 4:8]  # [1024, 4]
        blk = np.broadcast_to(erv[:, None, :], (len(rows), Tw, 4))
        er_lay[c_of[rows], p_of[rows],
               a_off[w]:a_off[w] + 4 * Tw] = blk.reshape(len(rows), 4 * Tw)
    # s-floor slot at t=deg
    has_pad = deg < Ts_np[k_of]
    rb_n = (c_of * P + p_of) * A4
    fcols = (rb_n + a_off[k_of] + deg * 4)[:, None] + hidx[None, :]
    el_lay.reshape(-1)[fcols[has_pad]] = FLOOR_LOGIT
    # zero er at the floor slot so lg = FLOOR exactly
    er_lay.reshape(-1)[fcols[has_pad]] = 0.0
    del fcols

    ident = np.eye(P, dtype=np.float32)
    in2 = []
    for c in range(NCORES):
        in2.append({"q": q_lay[c], "el": el_lay[c], "er": er_lay[c],
                    "sg": sg_lay[c], "ident": ident,
                    "bias": bias.reshape(1, HD)})
    res2 = run_bass_kernel_spmd(nc2, in2, core_ids=list(range(NCORES)),
                                trace=_TRACE)
    LAST_EXEC_NS2 = res2.exec_time_ns
    if LAST_EXEC_NS1 is not None and LAST_EXEC_NS2 is not None:
        LAST_EXEC_NS = LAST_EXEC_NS1 + LAST_EXEC_NS2
    out_full = np.zeros((N_PAD, D), dtype=np.float32)
    res_arr = np.stack([r["out"].reshape(P, K_WIN, D)
                        for r in res2.results])
    out_full[nodes] = res_arr[c_of, p_of, sched_pos[k_of], :]
    return np.ascontiguousarray(out_full[:N])


# revision 10
# speedup vs baseline: 1.0939x; 1.0939x over previous
"""AggrGATConv Trainium2 kernel v4: int16 message stream, globally-chunked
weight chain, engine-split identity aggregation.

Per-core (dst-sharded identity layout):
  inv-1: h = feat @ W (fp32 2-pass exact); q = round(h * 32767/max|h|) int16;
    sg = max|h|/(32767*H) (1/H head-mean folded in); el/er tables.
  host (index/data movement only): deg-sorted octet slotting; gathers
    q16[src] into per-window blobs, el[src]/sg[src] streams, er replicated
    per slot (erx).  All weight streams (t,h)-major per window.
  inv-2:
    global chain (chunked contiguous ops): lg = el + erx; e1 = exp(lg);
      e2 = exp(.2 lg); ee = max(e1,e2); esg = ee * sg
    per T-group: s4 = sum_t ee; r4 = 1/s4; es = esg * r4_bc
    per subgroup (<=4 same-T windows): one q DMA; PE-red subgroups get one
      merged mult (DVE or GpSimd); DVE-red windows get per-window mults.
    reduce: PE 1-tile identity matmuls + [P,D,H] X-reduce,  or fused
      XY-reduce on DVE.  Outputs accumulate in outbuf; one +bias; one DMA.
"""
import sys
import types
import contextlib
import ctypes
import os

import numpy as np

import concourse.bacc as bacc
import concourse.tile as tile
import concourse.mybir as mybir
from concourse.bass_utils import run_bass_kernel_spmd

# ---------------- constants (hardcoded per problem spec) ----------------
N = 100000
E = 1600000
IN = 128
H, D = 4, 32
HD = H * D  # 128
NEG = 0.2
NCORES = 8
P = 128
K_WIN = 98
N_PAD = NCORES * K_WIN * P       # 100352
NODES_PER_CORE = K_WIN * P       # 12544
PAD_LOGIT = -100.0
FLOOR_LOGIT = -69.07755  # 5*ln(1e-6): keeps s4 >= 1e-6 on padded rows
QMAX = 32767.0

SUBG_TILES = 20     # max total tiles per subgroup
PE_RED_FRAC = 0.68  # fraction of windows with PE reduce
GPS_MULT_FRAC = 0.60  # fraction of windows with GpSimd mult
CHAIN_CHUNK = 700   # target elems/partition per global-chain chunk

f32 = mybir.dt.float32
i16 = mybir.dt.int16

Exp = mybir.ActivationFunctionType.Exp
Copy = mybir.ActivationFunctionType.Copy
Add = mybir.AluOpType.add
Mult = mybir.AluOpType.mult
Max = mybir.AluOpType.max


def _install_ntff_shim():
    if "antenv.axon_hooks" in sys.modules:
        return
    try:
        lib = ctypes.CDLL("/opt/axon/libaxon_pjrt.so")
        if not hasattr(lib, "axon_start_nrt_profile"):
            raise OSError("no symbol")
        lib.axon_start_nrt_profile.argtypes = [
            ctypes.POINTER(ctypes.c_int64), ctypes.c_size_t]
        lib.axon_start_nrt_profile.restype = ctypes.c_int64
        lib.axon_stop_nrt_profile.argtypes = [ctypes.c_char_p]
        lib.axon_stop_nrt_profile.restype = ctypes.c_int64

        @contextlib.contextmanager
        def _hook(output_dir, device_ids):
            import jax
            jax.devices()
            if device_ids:
                ids = (ctypes.c_int64 * len(device_ids))(*device_ids)
                rc = lib.axon_start_nrt_profile(ids, len(device_ids))
            else:
                rc = lib.axon_start_nrt_profile(None, 0)
            if rc != 0:
                raise RuntimeError(f"axon_start_nrt_profile rc={rc}")
            try:
                yield
            finally:
                n = lib.axon_stop_nrt_profile(str(output_dir).encode())
                print(f"profile: {n} file(s) -> {output_dir}", file=sys.stderr)

        hook = _hook
    except OSError:
        hook = None
    mod = types.ModuleType("antenv.axon_hooks")
    mod.get_axon_ntff_profile_hook = lambda: hook
    mod.set_axon_ntff_profile_hook = lambda h: None
    sys.modules["antenv.axon_hooks"] = mod


_install_ntff_shim()


# ---------------- invocation 1: node tables + int16 quantization ---------
def _build_inv1():
    nc = bacc.Bacc("TRN2", target_bir_lowering=False, debug=False,
                   num_devices=NCORES)
    featT = nc.declare_dram_parameter("featT", [P, NODES_PER_CORE], f32,
                                      isOutput=False)
    W_in = nc.declare_dram_parameter("W", [IN, HD], f32, isOutput=False)
    WT_in = nc.declare_dram_parameter("WT", [HD, IN], f32, isOutput=False)
    Al_in = nc.declare_dram_parameter("Al", [HD, 4], f32, isOutput=False)
    Ar_in = nc.declare_dram_parameter("Ar", [HD, 4], f32, isOutput=False)
    q_out = nc.declare_dram_parameter("q_out", [P, K_WIN * HD], i16,
                                      isOutput=True)
    elr_out = nc.declare_dram_parameter("elr_out", [P, K_WIN * 8], f32,
                                        isOutput=True)
    sg_out = nc.declare_dram_parameter("sg_out", [P, K_WIN * 4], f32,
                                       isOutput=True)

    with tile.TileContext(nc) as tc:
        with tc.tile_pool(name="cst", bufs=1) as cst, \
             tc.tile_pool(name="sb", bufs=3) as sb, \
             tc.tile_pool(name="ps", bufs=3, space="PSUM") as ps, \
             tc.tile_pool(name="psw", bufs=1, space="PSUM") as psw:

            wt_sb = cst.tile([HD, IN], f32, tag="wt")
            nc.sync.dma_start(out=wt_sb[:], in_=WT_in[:])
            al_sb = cst.tile([HD, 4], f32, tag="al")
            nc.sync.dma_start(out=al_sb[:], in_=Al_in[:])
            ar_sb = cst.tile([HD, 4], f32, tag="ar")
            nc.sync.dma_start(out=ar_sb[:], in_=Ar_in[:])

            wlr = cst.tile([IN, 136], f32, tag="wlr")
            nc.sync.dma_start(out=wlr[:, 0:HD], in_=W_in[:])
            wl_ps = psw.tile([IN, 8], f32, tag="wlp")
            nc.tensor.matmul(out=wl_ps[:, 0:4], lhsT=wt_sb[:], rhs=al_sb[:],
                             start=True, stop=True)
            nc.tensor.matmul(out=wl_ps[:, 4:8], lhsT=wt_sb[:], rhs=ar_sb[:],
                             start=True, stop=True)
            nc.scalar.activation(out=wlr[:, 128:136], in_=wl_ps[:],
                                 func=Copy)

            CH = 14
            n_chunks = NODES_PER_CORE // (P * CH)
            for c in range(n_chunks):
                ft = sb.tile([P, CH * P], f32, tag="ft")
                nc.sync.dma_start(
                    out=ft[:], in_=featT[:, c * CH * P:(c + 1) * CH * P])
                hsb = sb.tile([P, CH * 136], f32, tag="hsb")
                for t in range(CH):
                    hp = ps.tile([P, 136], f32, tag="hp")
                    nc.tensor.matmul(out=hp[:],
                                     lhsT=ft[:, t * P:(t + 1) * P],
                                     rhs=wlr[:], start=True, stop=True)
                    if t % 2 == 0:
                        nc.scalar.activation(
                            out=hsb[:, t * 136:(t + 1) * 136], in_=hp[:],
                            func=Copy)
                    else:
                        nc.vector.tensor_copy(hsb[:, t * 136:(t + 1) * 136],
                                              hp[:])
                ga = hsb[:].rearrange("p (c f) -> p c f", c=CH)
                hview = ga[:, :, 0:128].rearrange(
                    "p c (hh d) -> p c hh d", hh=H)
                m4 = sb.tile([P, CH * 4], f32, tag="m4")
                nc.vector.tensor_reduce(
                    out=m4[:].rearrange("p (c h) -> p c h", c=CH),
                    in_=hview, axis=mybir.AxisListType.X, op=Max,
                    apply_absolute_value=True)
                # sg table carries the 1/H fold for the head-mean
                sgc = sb.tile([P, CH * 4], f32, tag="sgc")
                nc.vector.tensor_scalar_mul(sgc[:], m4[:], 1.0 / (QMAX * H))
                minv = sb.tile([P, CH * 4], f32, tag="minv")
                nc.vector.reciprocal(minv[:], m4[:])
                sinv = sb.tile([P, CH * 4], f32, tag="sinv")
                nc.vector.tensor_scalar_mul(sinv[:], minv[:], QMAX)
                q16 = sb.tile([P, CH * HD], i16, tag="q16")
                nc.vector.tensor_tensor(
                    out=q16[:].rearrange("p (c hh d) -> p c hh d",
                                         c=CH, hh=H),
                    in0=hview,
                    in1=sinv[:].rearrange("p (c h) -> p c h", c=CH)
                        .unsqueeze(3).to_broadcast([P, CH, H, D]),
                    op=Mult)
                elrs = sb.tile([P, CH * 8], f32, tag="elrs")
                nc.gpsimd.tensor_copy(
                    elrs[:].rearrange("p (c e) -> p c e", c=CH),
                    ga[:, :, 128:136])
                nc.gpsimd.dma_start(
                    out=q_out[:, c * CH * HD:(c + 1) * CH * HD], in_=q16[:])
                nc.gpsimd.dma_start(
                    out=elr_out[:, c * CH * 8:(c + 1) * CH * 8], in_=elrs[:])
                nc.gpsimd.dma_start(
                    out=sg_out[:, c * CH * 4:(c + 1) * CH * 4], in_=sgc[:])
    nc.compile()
    return nc


def id_key(ws):
    return tuple(ws)


def _schedule(Ts):
    """ASC-T schedule with equal-T groups, subgroups, engine classes and
    stream offsets.  All layouts (t,h)-major per window."""
    Ts = list(Ts)
    order = [int(x) for x in np.argsort(np.asarray(Ts), kind="stable")]
    groups = []
    i = 0
    while i < len(order):
        j = i
        tval = Ts[order[i]]
        while j < len(order) and Ts[order[j]] == tval:
            j += 1
        groups.append((int(tval), order[i:j]))
        i = j
    # stream offsets in scheduled order
    q_off, a_off = {}, {}
    qo = ao = 0
    for w in order:
        q_off[w] = qo
        qo += Ts[w] * HD
        a_off[w] = ao
        ao += 4 * Ts[w]
    A4 = ao
    # subgroups + engine classes (window-count-weighted ratio targets)
    sub_of = []           # list of (T, [windows], use_pe, use_gps)
    npe = ngps = ntot = 0
    for (tval, ws) in groups:
        k = 0
        nwin = max(1, SUBG_TILES // tval)
        while k < len(ws):
            sub = ws[k:k + nwin]
            k += nwin
            use_pe = (npe <= PE_RED_FRAC * ntot)
            use_gps = (ngps <= GPS_MULT_FRAC * ntot)
            ntot += len(sub)
            if use_pe:
                npe += len(sub)
            if use_gps:
                ngps += len(sub)
            sub_of.append((tval, sub, use_pe, use_gps))
    use_pe_w = {}
    for (tval, sub, upe, ugps) in sub_of:
        for w in sub:
            use_pe_w[w] = upe
    # chain chunks: consecutive groups totalling <= CHAIN_CHUNK elems/part
    chunks = []
    cur = []
    cur_elems = 0
    for (tval, ws) in groups:
        gsz = 4 * tval * len(ws)
        if cur and cur_elems + gsz > CHAIN_CHUNK:
            chunks.append(cur)
            cur = []
            cur_elems = 0
        cur.append((tval, ws))
        cur_elems += gsz
    if cur:
        chunks.append(cur)
    # map each subgroup to its chunk index (subgroups never span groups)
    chunk_of_group = {}
    for ci, ch in enumerate(chunks):
        for (tval, ws) in ch:
            chunk_of_group[id_key(ws)] = ci
    sub_chunks = [[] for _ in chunks]
    gkey = {}
    for ci, ch in enumerate(chunks):
        for (tval, ws) in ch:
            for w in ws:
                gkey[w] = ci
    for sub in sub_of:
        sub_chunks[gkey[sub[1][0]]].append(sub)
    return dict(order=order, groups=groups, q_off=q_off, a_off=a_off,
                A4=A4, CAPQ=qo, subs=sub_of, use_pe=use_pe_w,
                chunks=chunks, sub_chunks=sub_chunks)


# ---------------- invocation 2: edge aggregation ----------------
def _build_inv2(Ts):
    meta = _schedule(Ts)
    order = meta["order"]
    q_off, a_off, A4, CAPQ = (meta["q_off"], meta["a_off"], meta["A4"],
                              meta["CAPQ"])
    nsched = len(order)
    sched_pos = {w: i for i, w in enumerate(order)}

    nc = bacc.Bacc("TRN2", target_bir_lowering=False, debug=False,
                   num_devices=NCORES)
    q_d = nc.declare_dram_parameter("q", [P, CAPQ], i16, isOutput=False)
    el_d = nc.declare_dram_parameter("el", [P, A4], f32, isOutput=False)
    er_d = nc.declare_dram_parameter("er", [P, A4], f32, isOutput=False)
    sg_d = nc.declare_dram_parameter("sg", [P, A4], f32, isOutput=False)
    ident_d = nc.declare_dram_parameter("ident", [P, P], f32, isOutput=False)
    bias_in = nc.declare_dram_parameter("bias", [1, HD], f32, isOutput=False)
    out_d = nc.declare_dram_parameter("out", [P, K_WIN * D], f32,
                                      isOutput=True)

    with tile.TileContext(nc) as tc:
        with tc.tile_pool(name="cst", bufs=1) as cst, \
             tc.tile_pool(name="ax", bufs=2) as ax, \
             tc.tile_pool(name="ch", bufs=2) as chp, \
             tc.tile_pool(name="ld", bufs=3) as ld, \
             tc.tile_pool(name="wk", bufs=2) as wk, \
             tc.tile_pool(name="fl", bufs=2) as fl, \
             tc.tile_pool(name="ps", bufs=6, space="PSUM") as ps, \
             tc.tile_pool(name="psb", bufs=1, space="PSUM") as psb:

            ident = cst.tile([P, P], f32, tag="ident")
            nc.sync.dma_start(out=ident[:], in_=ident_d[:])

            bias_sb = cst.tile([1, HD], f32, tag="brow")
            nc.sync.dma_start(out=bias_sb[:], in_=bias_in[:])
            bias_m = cst.tile([1, D], f32, tag="bm")
            nc.vector.tensor_reduce(
                out=bias_m[:],
                in_=bias_sb[0:1, :].rearrange("p (h d) -> p d h", h=H),
                axis=mybir.AxisListType.X, op=Add)
            nc.vector.tensor_scalar_mul(bias_m[:], bias_m[:], 1.0 / H)
            ones1 = cst.tile([1, P], f32, tag="ones")
            nc.vector.memset(ones1[:], 1.0)
            bias_ps = psb.tile([P, D], f32, tag="bps")
            nc.tensor.matmul(out=bias_ps[:], lhsT=ones1[:], rhs=bias_m[:],
                             start=True, stop=True)
            bias_bc = cst.tile([P, D], f32, tag="bbc")
            nc.vector.tensor_copy(bias_bc[:], bias_ps[:])

            outbuf = cst.tile([P, nsched * D], f32, tag="outbuf")

            # per-chunk: weight chain then that chunk's subgroups
            for ci, chunk in enumerate(meta["chunks"]):
                c_lo = a_off[chunk[0][1][0]]
                c_sz = sum(4 * t * len(ws) for (t, ws) in chunk)
                elt = ax.tile([P, c_sz], f32, tag="elt")
                nc.sync.dma_start(out=elt[:], in_=el_d[:, c_lo:c_lo + c_sz])
                ert = ax.tile([P, c_sz], f32, tag="ert")
                nc.sync.dma_start(out=ert[:], in_=er_d[:, c_lo:c_lo + c_sz])
                sgt = ax.tile([P, c_sz], f32, tag="sgt")
                nc.sync.dma_start(out=sgt[:], in_=sg_d[:, c_lo:c_lo + c_sz])

                lg = chp.tile([P, c_sz], f32, tag="lg")
                nc.vector.tensor_tensor(out=lg[:], in0=elt[:], in1=ert[:],
                                        op=Add)
                e1 = chp.tile([P, c_sz], f32, tag="e1")
                nc.scalar.activation(out=e1[:], in_=lg[:], func=Exp)
                e2 = chp.tile([P, c_sz], f32, tag="e2")
                nc.scalar.activation(out=e2[:], in_=lg[:], scale=NEG,
                                     func=Exp)
                ee = chp.tile([P, c_sz], f32, tag="ee")
                nc.vector.tensor_tensor(out=ee[:], in0=e1[:], in1=e2[:],
                                        op=Max)
                esg = chp.tile([P, c_sz], f32, tag="esg")
                nc.vector.tensor_tensor(out=esg[:], in0=ee[:], in1=sgt[:],
                                        op=Mult)
                es = chp.tile([P, c_sz], f32, tag="es")

                for (T, ws) in chunk:
                    nk = len(ws)
                    g0 = a_off[ws[0]] - c_lo
                    gsz = 4 * T * nk
                    s4 = fl.tile([P, nk * 4], f32, tag="s4")
                    nc.vector.tensor_reduce(
                        out=s4[:].rearrange("p (k h) -> p k h", k=nk),
                        in_=ee[:, g0:g0 + gsz].rearrange(
                            "p (k t h) -> p k h t", k=nk, h=H),
                        axis=mybir.AxisListType.X, op=Add)
                    r4 = fl.tile([P, nk * 4], f32, tag="r4")
                    nc.vector.reciprocal(r4[:], s4[:])
                    nc.vector.tensor_tensor(
                        out=es[:, g0:g0 + gsz].rearrange(
                            "p (k t h) -> p k t h", k=nk, h=H),
                        in0=esg[:, g0:g0 + gsz].rearrange(
                            "p (k t h) -> p k t h", k=nk, h=H),
                        in1=r4[:].rearrange("p (k h) -> p k h", k=nk)
                            .unsqueeze(2).to_broadcast([P, nk, T, H]),
                        op=Mult)

                # ---- this chunk's subgroups ----
                for (T, sub, use_pe, use_gps) in meta["sub_chunks"][ci]:
                    ns = len(sub)
                    KW = T * HD
                    qt = ld.tile([P, ns * KW], i16, tag="qt")
                    nc.sync.dma_start(
                        out=qt[:],
                        in_=q_d[:, q_off[sub[0]]:q_off[sub[0]] + ns * KW])
                    e0 = a_off[sub[0]] - c_lo
                    if use_pe:
                        # merged mult across the subgroup; layout (t,h,d)
                        meng = nc.gpsimd if use_gps else nc.vector
                        wmsg = wk.tile([P, ns * KW], f32, tag="wmsg")
                        meng.tensor_tensor(
                            out=wmsg[:].rearrange(
                                "p (kt hh d) -> p kt hh d", hh=H, d=D),
                            in0=qt[:].rearrange(
                                "p (kt hh d) -> p kt hh d", hh=H, d=D),
                            in1=es[:, e0:e0 + ns * 4 * T]
                                .rearrange("p (kt h) -> p kt h", h=H)
                                .unsqueeze(3).to_broadcast(
                                    [P, ns * T, H, D]),
                            op=Mult)
                        for si, w in enumerate(sub):
                            u = ps.tile([P, HD], f32, tag="u")
                            for t in range(T):
                                nc.tensor.matmul(
                                    out=u[:], lhsT=ident[:],
                                    rhs=wmsg[:, (si * T + t) * HD:
                                             (si * T + t + 1) * HD],
                                    start=(t == 0), stop=(t == T - 1))
                            sp = sched_pos[w]
                            nc.vector.tensor_reduce(
                                out=outbuf[:, sp * D:(sp + 1) * D],
                                in_=u[:].rearrange("p (hh d) -> p d hh",
                                                   hh=H),
                                axis=mybir.AxisListType.X, op=Add)
                    else:
                        # per-window mult (h,d,t) + fused XY reduce, DVE
                        for si, w in enumerate(sub):
                            wmsg = wk.tile([P, KW], f32, tag="wmsg")
                            nc.vector.tensor_tensor(
                                out=wmsg[:].rearrange(
                                    "p (hh d t) -> p hh d t", hh=H, d=D),
                                in0=qt[:, si * KW:(si + 1) * KW]
                                    .rearrange("p (hh d t) -> p hh d t",
                                               hh=H, d=D),
                                in1=es[:, e0 + si * 4 * T:
                                       e0 + (si + 1) * 4 * T]
                                    .rearrange("p (t h) -> p h t", h=H)
                                    .unsqueeze(2).to_broadcast(
                                        [P, H, D, T]),
                                op=Mult)
                            sp = sched_pos[w]
                            nc.vector.tensor_reduce(
                                out=outbuf[:, sp * D:(sp + 1) * D],
                                in_=wmsg[:].rearrange(
                                    "p (hh d t) -> p d hh t", hh=H, d=D),
                                axis=mybir.AxisListType.XY, op=Add)

            # ---- finalize ----
            nc.vector.tensor_tensor(
                out=outbuf[:].rearrange("p (k d) -> p k d", k=nsched),
                in0=outbuf[:].rearrange("p (k d) -> p k d", k=nsched),
                in1=bias_bc[:].unsqueeze(1).to_broadcast([P, nsched, D]),
                op=Add)
            nc.gpsimd.dma_start(out=out_d[:, 0:nsched * D], in_=outbuf[:])
    nc.compile()
    return nc, meta


_INV1 = None
_INV2 = {}
LAST_EXEC_NS = None
LAST_EXEC_NS1 = None
LAST_EXEC_NS2 = None
_TRACE = bool(os.environ.get("GAT_TRACE"))


def kernel(feat, W, attn_l, attn_r, bias, src, dst):
    global _INV1, LAST_EXEC_NS, LAST_EXEC_NS1, LAST_EXEC_NS2
    feat = np.asarray(feat, dtype=np.float32)
    W = np.asarray(W, dtype=np.float32)
    attn_l = np.asarray(attn_l, dtype=np.float32)
    attn_r = np.asarray(attn_r, dtype=np.float32)
    bias = np.asarray(bias, dtype=np.float32)
    src = np.asarray(src, dtype=np.int32)
    dst = np.asarray(dst, dtype=np.int32)

    featT = np.zeros((IN, N_PAD), dtype=np.float32)
    featT[:, :N] = np.ascontiguousarray(feat.T)
    WT = np.ascontiguousarray(W.T)
    Al = np.zeros((HD, H), dtype=np.float32)
    Ar = np.zeros((HD, H), dtype=np.float32)
    for h in range(H):
        Al[h * D:(h + 1) * D, h] = attn_l[h]
        Ar[h * D:(h + 1) * D, h] = attn_r[h]

    # ---------------- inv-1 ----------------
    if _INV1 is None:
        _INV1 = _build_inv1()
    in1 = []
    for c in range(NCORES):
        sl = slice(c * NODES_PER_CORE, (c + 1) * NODES_PER_CORE)
        in1.append({"featT": np.ascontiguousarray(featT[:, sl]),
                    "W": W, "WT": WT, "Al": Al, "Ar": Ar})
    res1 = run_bass_kernel_spmd(_INV1, in1, core_ids=list(range(NCORES)),
                                trace=_TRACE)
    LAST_EXEC_NS1 = res1.exec_time_ns
    q_full = np.concatenate(
        [r["q_out"].reshape(P, K_WIN, HD).transpose(1, 0, 2)
         .reshape(NODES_PER_CORE, HD) for r in res1.results], axis=0)
    elr_full = np.concatenate(
        [r["elr_out"].reshape(P, K_WIN, 8).transpose(1, 0, 2)
         .reshape(NODES_PER_CORE, 8) for r in res1.results], axis=0)
    sg_full = np.concatenate(
        [r["sg_out"].reshape(P, K_WIN, 4).transpose(1, 0, 2)
         .reshape(NODES_PER_CORE, 4) for r in res1.results], axis=0)

    # ---------------- host: identity-layout slotting ----------------
    deg = np.bincount(dst, minlength=N_PAD).astype(np.int64)
    order_n = np.argsort(-deg, kind="stable")
    rank = np.empty(N_PAD, dtype=np.int64)
    rank[order_n] = np.arange(N_PAD)
    k_of = rank >> 10
    within = rank & 1023
    c_of = within >> 7
    c_of = np.where(k_of & 1 == 1, NCORES - 1 - c_of, c_of)  # snake
    p_of = within & 127

    Ts = deg[order_n[::1024]]
    Ts = np.maximum(Ts, 1)
    key = tuple(int(t) for t in Ts)
    if key not in _INV2:
        _INV2[key] = _build_inv2(key)
    nc2, meta = _INV2[key]

    Ts_np = np.asarray(key, dtype=np.int64)
    nsched = len(meta["order"])
    sched_pos = np.empty(K_WIN, dtype=np.int64)
    sched_pos[np.asarray(meta["order"])] = np.arange(nsched)
    q_off = np.zeros(K_WIN, dtype=np.int64)
    a_off = np.zeros(K_WIN, dtype=np.int64)
    use_pe_w = np.zeros(K_WIN, dtype=bool)
    for w in range(K_WIN):
        q_off[w] = meta["q_off"][w]
        a_off[w] = meta["a_off"][w]
        use_pe_w[w] = meta["use_pe"][w]
    CAPQ, A4 = meta["CAPQ"], meta["A4"]

    # per-edge slots
    perm = np.argsort(dst, kind="stable")
    dstp = dst[perm]
    srcp = src[perm]
    estart = np.zeros(N_PAD + 1, dtype=np.int64)
    np.cumsum(np.bincount(dstp, minlength=N_PAD), out=estart[1:])
    te = np.arange(E, dtype=np.int64) - estart[dstp]
    ce = c_of[dstp]
    pe_row = p_of[dstp]
    we = k_of[dstp]
    Te = Ts_np[we]

    # ---- q stream ----
    q_lay = np.zeros((NCORES, P, CAPQ), dtype=np.int16)
    qflat = q_lay.reshape(-1)
    rowbase = (ce * P + pe_row) * CAPQ
    hdidx = np.arange(HD, dtype=np.int64)
    is_pe_e = use_pe_w[we]
    idx_pe = np.nonzero(is_pe_e)[0]
    idx_dv = np.nonzero(~is_pe_e)[0]
    # PE windows: (t, h, d) -> cols q_off + t*128 + hd
    cols = (rowbase[idx_pe] + q_off[we[idx_pe]]
            + te[idx_pe] * HD)[:, None] + hdidx[None, :]
    qflat[cols] = q_full[srcp[idx_pe]]
    del cols
    # DVE windows: (h, d, t) -> cols q_off + hd*T + t
    cols = (rowbase[idx_dv] + q_off[we[idx_dv]] + te[idx_dv])[:, None] \
        + hdidx[None, :] * Te[idx_dv][:, None]
    qflat[cols] = q_full[srcp[idx_dv]]
    del cols

    # ---- weight streams (t,h)-major ----
    el_lay = np.full((NCORES, P, A4), PAD_LOGIT, dtype=np.float32)
    er_lay = np.zeros((NCORES, P, A4), dtype=np.float32)
    sg_lay = np.zeros((NCORES, P, A4), dtype=np.float32)
    hidx = np.arange(H, dtype=np.int64)
    rb_a = (ce * P + pe_row) * A4
    ecols = (rb_a + a_off[we] + te * 4)[:, None] + hidx[None, :]
    el_lay.reshape(-1)[ecols] = elr_full[srcp][:, 0:4]
    sg_lay.reshape(-1)[ecols] = sg_full[srcp]
    del ecols
    # er replicated across all slots of each row (including pads: harmless)
    nodes = np.arange(N_PAD)
    for w in range(K_WIN):
        rows = nodes[k_of == w]
        Tw = int(Ts_np[w])
        erv = elr_full[rows][:, 4:8]  # [1024, 4]
        blk = np.broadcast_to(erv[:, None, :], (len(rows), Tw, 4))
        er_lay[c_of[rows], p_of[rows],
               a_off[w]:a_off[w] + 4 * Tw] = blk.reshape(len(rows), 4 * Tw)
    # s-floor slot at t=deg
    has_pad = deg < Ts_np[k_of]
    rb_n = (c_of * P + p_of) * A4
    fcols = (rb_n + a_off[k_of] + deg * 4)[:, None] + hidx[None, :]
    el_lay.reshape(-1)[fcols[has_pad]] = FLOOR_LOGIT
    # zero er at the floor slot so lg = FLOOR exactly
    er_lay.reshape(-1)[fcols[has_pad]] = 0.0
    del fcols

    ident = np.eye(P, dtype=np.float32)
    in2 = []
    for c in range(NCORES):
        in2.append({"q": q_lay[c], "el": el_lay[c], "er": er_lay[c],
                    "sg": sg_lay[c], "ident": ident,
                    "bias": bias.reshape(1, HD)})
    res2 = run_bass_kernel_spmd(nc2, in2, core_ids=list(range(NCORES)),
                                trace=_TRACE)
    LAST_EXEC_NS2 = res2.exec_time_ns
    if LAST_EXEC_NS1 is not None and LAST_EXEC_NS2 is not None:
        LAST_EXEC_NS = LAST_EXEC_NS1 + LAST_EXEC_NS2
    out_full = np.zeros((N_PAD, D), dtype=np.float32)
    res_arr = np.stack([r["out"].reshape(P, K_WIN, D)
                        for r in res2.results])
    out_full[nodes] = res_arr[c_of, p_of, sched_pos[k_of], :]
    return np.ascontiguousarray(out_full[:N])
